# revision 9
# baseline (speedup 1.0000x reference)
"""Causal self-attention (B=2, T=2048, E=1024, H=16, D=64) on 8 TRN2 NeuronCores.

Sharding: core = (batch b, head-group hg): 2 batches x 4 head-groups of 4 heads.

Design (fp32 PSUM everywhere; fp16 operands except the q/k path in fp8):
  - Q/K projections run in fp8 with DoubleRow pairs over the contraction
    (4 instructions instead of 16 per tile); x and the weights are supplied
    in both fp16 (for V) and fp8 (for Q/K).
  - q/k are stored fp8; scores use DoubleRow with a stride-0 slot broadcast,
    computing 2*(K^T Q) at half cost; the factor 2 folds into the exp scale.
  - attn@V in q-partition orientation: out [128q, 65] per (head, qt, kt) with
    a ones-column in V producing rowsums; matmul cost follows the free size
    (65), halving the PE time vs the transposed orientation.
  - Normalization: per-partition reciprocal + one strided broadcast multiply
    on DVE; no DRAM bounce.
  - attn-out transposed back via PE-transpose (eye) for the Wo projection.
  - Biases: bk dropped (softmax shift invariance), bv/bo folded into a host
    bias vector, bq fused into the Q PSUM-evacuation copy.
  - Emission is paced: score tiles feed the Activation engine continuously;
    all other PE work (projections, attn@V, transposes, Wo) interleaves
    between them with cost estimates so no engine starves.
"""
from contextlib import ExitStack

import numpy as np

import concourse.bass as bass  # noqa: F401
import concourse.mybir as mybir
import concourse.tile as tile
from concourse import bacc
from concourse.bass_utils import run_bass_kernel_spmd

T = 2048
E = 1024
HPC = 4          # heads per core
D = 64
S = HPC * D      # 256 per-core head-column slice
KE = E // 128    # 8 contraction tiles for the projections
NKT = T // 128   # 16 key row tiles
NTB = T // 512   # 4 T-blocks (projection granularity)
F16 = mybir.dt.float16
F32 = mybir.dt.float32
F8 = mybir.dt.float8e4
EXP = mybir.ActivationFunctionType.Exp
DR = mybir.MatmulPerfMode.DoubleRow
PE_CYC = 0.4167
import os
PACE_F = float(os.environ.get('PACE_F', '0.85'))
SCH_KTS = tuple(int(t) for t in os.environ.get('SCH_KTS', '2,6,10').split(',') if t)
READY_F = float(os.environ.get('READY_F', '1.0'))
SCH_MIN_QB = int(os.environ.get('SCH_MIN_QB', '1'))
I16 = mybir.dt.int16


def build_nc():
    nc = bacc.Bacc("TRN2", target_bir_lowering=False, debug=False)
    xT = nc.dram_tensor("xT", [E, T], F16, kind="ExternalInput").ap()
    x8 = nc.dram_tensor("x8", [E, T], F8, kind="ExternalInput").ap()
    wq8 = nc.dram_tensor("wq8", [E, S], F8, kind="ExternalInput").ap()
    wk8 = nc.dram_tensor("wk8", [E, S], F8, kind="ExternalInput").ap()
    wv = nc.dram_tensor("wv", [E, S], F16, kind="ExternalInput").ap()
    wo = nc.dram_tensor("wo", [S, E], F16, kind="ExternalInput").ap()
    bq = nc.dram_tensor("bq", [128, 2], F32, kind="ExternalInput").ap()
    mask = nc.dram_tensor("mask", [128, 128], F16, kind="ExternalInput").ap()
    eye = nc.dram_tensor("eye", [128, 128], F16, kind="ExternalInput").ap()
    out = nc.dram_tensor("out", [T, E], F16, kind="ExternalOutput").ap()

    with tile.TileContext(nc) as tc:
        _emit(nc, tc, xT, x8, wq8, wk8, wv, wo, bq, mask, eye, out)
    nc.compile()
    return nc


def _emit(nc, tc, xT, x8, wq8, wk8, wv, wo, bq, mask, eye, out):
    ctx = ExitStack()
    consts = ctx.enter_context(tc.tile_pool(name="consts", bufs=1))
    big_ps = ctx.enter_context(tc.tile_pool(name="big_ps", bufs=2, space="PSUM"))
    aux_ps = ctx.enter_context(tc.tile_pool(name="aux_ps", bufs=2, space="PSUM"))
    av_ps = ctx.enter_context(tc.tile_pool(name="av_ps", bufs=1, space="PSUM"))
    pt_pool = ctx.enter_context(tc.tile_pool(name="pt", bufs=28))
    sm_pool = ctx.enter_context(tc.tile_pool(name="sm", bufs=4))
    ob_pool = ctx.enter_context(tc.tile_pool(name="ob", bufs=3))

    # --- staging tiles ---
    x_sb = consts.tile([128, KE, T], F16)
    x8_sb = consts.tile([128, 4, 2, T], F8)
    wq8_sb = consts.tile([128, 4, 2, S], F8)
    wk8_sb = consts.tile([128, 4, 2, S], F8)
    wv_sb = consts.tile([128, KE, S], F16)
    wo_sb = consts.tile([128, 2, E], F16)
    bq_sb = consts.tile([128, 2], F32)
    mask_sb = consts.tile([128, 128], F16)
    eye_sb = consts.tile([128, 128], F16)
    qt_sb = consts.tile([128, 2, T], F8)
    kt_sb = consts.tile([128, 2, T], F8)
    v_sb = consts.tile([128, NKT, HPC, D + 1], F16)
    a_sb = consts.tile([128, NKT, HPC, D], F16)
    atT_sb = consts.tile([128, 2, T], F16)

    # load order: q/k-projection needs first, V path behind it
    x8r = x8.rearrange("(kp two p) n -> p kp two n", p=128, two=2)
    xr = xT.rearrange("(k p) n -> p k n", p=128)
    wq8r = wq8.rearrange("(kp two p) m -> p kp two m", p=128, two=2)
    wk8r = wk8.rearrange("(kp two p) m -> p kp two m", p=128, two=2)
    nc.sync.dma_start(out=wq8_sb[:, :, :, 0:128], in_=wq8r[:, :, :, 0:128])
    nc.scalar.dma_start(out=x8_sb[:, :, :, 0:512], in_=x8r[:, :, :, 0:512])
    nc.sync.dma_start(out=wk8_sb[:, :, :, 0:128], in_=wk8r[:, :, :, 0:128])
    nc.gpsimd.dma_start(out=bq_sb, in_=bq)
    nc.gpsimd.dma_start(out=mask_sb, in_=mask)
    nc.scalar.dma_start(out=x8_sb[:, :, :, 512:1024], in_=x8r[:, :, :, 512:1024])
    nc.scalar.dma_start(out=wq8_sb[:, :, :, 128:256], in_=wq8r[:, :, :, 128:256])
    nc.sync.dma_start(out=wk8_sb[:, :, :, 128:256], in_=wk8r[:, :, :, 128:256])
    nc.gpsimd.dma_start(out=wv_sb, in_=wv.rearrange("(k p) m -> p k m", p=128))
    nc.sync.dma_start(out=x_sb[:, :, 0:512], in_=xr[:, :, 0:512])
    nc.scalar.dma_start(out=x8_sb[:, :, :, 1024:1536], in_=x8r[:, :, :, 1024:1536])
    nc.gpsimd.dma_start(out=x_sb[:, :, 512:1024], in_=xr[:, :, 512:1024])
    nc.sync.dma_start(out=x8_sb[:, :, :, 1536:2048], in_=x8r[:, :, :, 1536:2048])
    nc.scalar.dma_start(out=x_sb[:, :, 1024:1536], in_=xr[:, :, 1024:1536])
    nc.gpsimd.dma_start(out=x_sb[:, :, 1536:2048], in_=xr[:, :, 1536:2048])
    nc.scalar.dma_start(out=eye_sb, in_=eye)
    nc.sync.dma_start(out=wo_sb, in_=wo.rearrange("(a p) n -> p a n", p=128))
    nc.vector.memset(v_sb[:, :, :, D : D + 1], 1.0)

    # ---------- emit helpers ----------
    def emit_q_half(fb, tb):
        ts = slice(tb * 512, (tb + 1) * 512)
        ps = aux_ps.tile([128, 512], F32, tag="x", name=f"q{fb}_{tb}")
        for kp in range(4):
            nc.tensor.matmul(
                ps,
                lhsT=wq8_sb[:, kp, :, fb * 128 : (fb + 1) * 128],
                rhs=x8_sb[:, kp, :, ts],
                start=(kp == 0),
                stop=(kp == 3),
                perf_mode=DR,
            )
        with nc.allow_low_precision(reason="q stored fp8 for DR scores"):
            nc.vector.tensor_scalar_add(qt_sb[:, fb, ts], ps,
                                        bq_sb[:, fb : fb + 1])

    def emit_k_half(fb, tb):
        ts = slice(tb * 512, (tb + 1) * 512)
        ps = aux_ps.tile([128, 512], F32, tag="x", name=f"k{fb}_{tb}")
        for kp in range(4):
            nc.tensor.matmul(
                ps,
                lhsT=wk8_sb[:, kp, :, fb * 128 : (fb + 1) * 128],
                rhs=x8_sb[:, kp, :, ts],
                start=(kp == 0),
                stop=(kp == 3),
                perf_mode=DR,
            )
        with nc.allow_low_precision(reason="k stored fp8 for DR scores"):
            nc.vector.tensor_copy(kt_sb[:, fb, ts], ps)

    v_ps = {}

    def emit_v1kt(kt):
        m, r = divmod(kt, 2)
        if r == 0:
            v_ps[m] = aux_ps.tile([128, 512], F32, tag="x", name=f"v{m}")
        ps = v_ps[m] if r == 0 else v_ps.get(m)
        qs = slice(r * 256, (r + 1) * 256)
        for ke in range(KE):
            nc.tensor.matmul(
                ps[:, qs],
                lhsT=x_sb[:, ke, kt * 128 : (kt + 1) * 128],
                rhs=wv_sb[:, ke, :],
                start=(ke == 0),
                stop=(ke == KE - 1),
            )
        if r == 1:
            ps = v_ps.pop(m)
            nc.vector.tensor_copy(
                v_sb[:, 2 * m : 2 * m + 2, :, 0:D],
                ps.rearrange("p (kt h d) -> p kt h d", kt=2, h=HPC),
            )

    def emit_score(j, qb, kt):
        """scores (fp8 DR, 2*K^T Q) + exp (+ diagonal mask) for one key tile."""
        r = kt - 4 * qb
        off = 128 * r if r >= 1 else 0
        st = big_ps.tile([128, 1024], F32, tag="b", name=f"st{j}_{qb}_{kt}")
        for hh in range(2):
            hs = slice(hh * 64, (hh + 1) * 64)
            ktv = kt_sb[hs, j, kt * 128 : (kt + 1) * 128]
            qtv = qt_sb[hs, j, qb * 512 + off : (qb + 1) * 512]
            nc.tensor.matmul(
                st[:, hh * 512 + off : (hh + 1) * 512],
                lhsT=ktv.rearrange("p (o n) -> p o n", o=1).to_broadcast((64, 2, 128)),
                rhs=qtv.rearrange("p (o n) -> p o n", o=1).to_broadcast((64, 2, 512 - off)),
                start=True,
                stop=True,
                perf_mode=DR,
            )
        pt = pt_pool.tile([128, 2, 512], F16, tag="pt", name=f"pt{j}_{qb}_{kt}")
        stv = st.rearrange("p (a n) -> p a n", a=2)
        if r < 0 and kt in SCH_KTS and qb >= SCH_MIN_QB:
            # Schraudolph exp on DVE: int16 pattern = round(2s*0.0625*1024/ln2
            # + 15*1024 - 59.3), bitcast to fp16 (max rel err ~3%, washes out
            # under the fp8 q/k error; frees the Act engine)
            nc.vector.tensor_scalar(pt.bitcast(I16), stv,
                                    92.332483, 15300.7,
                                    op0=mybir.AluOpType.mult,
                                    op1=mybir.AluOpType.add)
        elif off:
            nc.scalar.activation(pt[:, :, off:], stv[:, :, off:], EXP, scale=0.0625)
        else:
            nc.scalar.activation(pt, stv, EXP, scale=0.0625)
        if r >= 0:
            nc.vector.tensor_mul(
                pt[:, :, off : off + 128],
                pt[:, :, off : off + 128],
                mask_sb.rearrange("p (o n) -> p o n", o=1).to_broadcast((128, 2, 128)),
            )
        return pt

    av_tiles = {}

    def emit_att_hqt(j, qb, hh, qt):
        """attn@V for one (head, q-tile); norm for the head on the last qt."""
        pts = row_pts[(j, qb)]
        if (j, qb) not in av_tiles:
            av_tiles[(j, qb)] = av_ps.tile(
                [128, 8, 128], F32, tag="av", name=f"av{j}_{qb}"
            )
        av = av_tiles[(j, qb)]
        h = 2 * j + hh
        qtg = 4 * qb + qt
        for kt in range(qtg + 1):
            nc.tensor.matmul(
                av[:, hh * 4 + qt, 0 : D + 1],
                lhsT=pts[kt][:, hh, qt * 128 : (qt + 1) * 128],
                rhs=v_sb[:, kt, h, :],
                start=(kt == 0),
                stop=(kt == qtg),
            )
        if qt == 3:
            rr = sm_pool.tile([128, 4], F32, tag="rr", name=f"rr{j}_{qb}_{hh}")
            nc.vector.reciprocal(
                rr, av[:, hh * 4 : hh * 4 + 4, D : D + 1].rearrange("p a o -> p (a o)")
            )
            nc.vector.tensor_mul(
                a_sb[:, 4 * qb : 4 * qb + 4, h, :],
                av[:, hh * 4 : hh * 4 + 4, 0:D],
                rr.rearrange("p (a o) -> p a o", o=1).to_broadcast((128, 4, D)),
            )

    def emit_att_qt(j, qb, qt):
        av = av_tiles[(j, qb)]
        pts = row_pts[(j, qb)]
        qtg = 4 * qb + qt
        for hh in range(2):
            h = 2 * j + hh
            for kt in range(qtg + 1):
                nc.tensor.matmul(
                    av[:, hh * 4 + qt, 0 : D + 1],
                    lhsT=pts[kt][:, hh, qt * 128 : (qt + 1) * 128],
                    rhs=v_sb[:, kt, h, :],
                    start=(kt == 0),
                    stop=(kt == qtg),
                )
        rr = sm_pool.tile([128, 2, 1], F32, tag="rr2", name=f"rq{j}_{qb}_{qt}")
        nc.vector.reciprocal(rr, av[:, qt : qt + 5 : 4, D : D + 1])
        nc.vector.tensor_mul(
            a_sb[:, qtg, 2 * j : 2 * j + 2, :],
            av[:, qt : qt + 5 : 4, 0:D],
            rr.to_broadcast((128, 2, D)),
        )

    def emit_tp_qt(qtg, on_act=False):
        tp = aux_ps.tile([128, 2, 128], F16, tag="x", name=f"tpq{qtg}")
        for sb in range(2):
            nc.tensor.matmul(
                tp[:, sb, :],
                lhsT=a_sb[:, qtg, 2 * sb : 2 * sb + 2, :].rearrange(
                    "p a d -> p (a d)"
                ),
                rhs=eye_sb,
                start=True,
                stop=True,
                is_transpose=True,
            )
        if on_act:
            nc.scalar.copy(atT_sb[:, :, qtg * 128 : (qtg + 1) * 128], tp)
        else:
            nc.vector.tensor_copy(
                atT_sb[:, :, qtg * 128 : (qtg + 1) * 128], tp
            )

    def emit_tp(qb):
        tp = aux_ps.tile([128, 8, 128], F16, tag="x", name=f"tp{qb}")
        for qt in range(4):
            for sb in range(2):
                nc.tensor.matmul(
                    tp[:, qt * 2 + sb, :],
                    lhsT=a_sb[:, 4 * qb + qt, 2 * sb : 2 * sb + 2, :].rearrange(
                        "p a d -> p (a d)"
                    ),
                    rhs=eye_sb,
                    start=True,
                    stop=True,
                    is_transpose=True,
                )
        dst = atT_sb.rearrange("p a (q n) -> p q a n", n=128)[:, 4 * qb : 4 * qb + 4]
        nc.vector.tensor_copy(dst, tp.rearrange("p (q a) n -> p q a n", a=2))

    def emit_wo_half(qtg, eh, on_act=False):
        ps = aux_ps.tile([128, 512], F32, tag="x", name=f"wops{qtg}_{eh}")
        for sb in range(2):
            nc.tensor.matmul(
                ps,
                lhsT=atT_sb[:, sb, qtg * 128 : (qtg + 1) * 128],
                rhs=wo_sb[:, sb, eh * 512 : (eh + 1) * 512],
                start=(sb == 0),
                stop=(sb == 1),
            )
        ob = ob_pool.tile([128, 512], F16, tag="ob", name=f"ob{qtg}_{eh}")
        if on_act:
            nc.scalar.copy(ob, ps)
        else:
            nc.vector.tensor_copy(ob, ps)
        nc.sync.dma_start(out=out[qtg * 128 : (qtg + 1) * 128, eh * 512 : (eh + 1) * 512],
                          in_=ob)

    def emit_wo_qt(qtg):
        emit_wo_half(qtg, 0)
        emit_wo_half(qtg, 1)

    # ---------- paced interleaved emission ----------    # ---------- paced interleaved emission ----------
    row_pts = {}

    def att_units(j, qb):
        n = sum(4 * qb + qt + 1 for qt in range(4))
        return [(f"att{j}{qb}", (lambda hh=hh: emit_att_h(j, qb, hh)),
                 n * 65 * PE_CYC, (j, qb)) for hh in (0, 1)]

    def wo_units(qb):
        return [(f"wo{qb}", (lambda q=qtg: emit_wo_qt(q)), 860, None)
                for qtg in range(4 * qb, 4 * qb + 4)]

    fillers = []

    def F(group, thunk, ns, req=None, ready=0.0, after=()):
        ready *= READY_F
        fillers.append([group, thunk, ns, req, ready, False, tuple(after)])

    def att_units(j, qb):
        for hh in (0, 1):
            for qt in range(4):
                n = 4 * qb + qt + 1
                F(f"att{j}{qb}",
                  (lambda hh=hh, qt=qt: emit_att_hqt(j, qb, hh, qt)),
                  n * 65 * PE_CYC, (j, qb),
                  after=tuple(f"v{t}" for t in range(qb + 1)))

    def qk_units(fb, tb, ready=0.0):
        F(f"qk{fb}{tb}", lambda: emit_q_half(fb, tb), 430, ready=ready)
        F(f"qk{fb}{tb}", lambda: emit_k_half(fb, tb), 430, ready=ready)

    def v_units(tb, ready=0.0):
        for r in range(4):
            F(f"v{tb}", (lambda kt=4 * tb + r: emit_v1kt(kt)), 856, ready=ready)

    def wo_units(qb):
        for qtg in range(4 * qb, 4 * qb + 4):
            F(f"wo{qb}", (lambda q=qtg: emit_wo_half(q, 0)), 430, after=(f"tp{qb}",))
            F(f"wo{qb}", (lambda q=qtg: emit_wo_half(q, 1)), 430, after=(f"tp{qb}",))

    qk_units(0, 1, ready=12500)
    qk_units(1, 0, ready=11000)
    qk_units(1, 1, ready=11000)
    v_units(0, ready=15000)
    qk_units(0, 2, ready=19000)
    qk_units(1, 2, ready=19000)
    v_units(1, ready=22000)
    att_units(0, 0)
    qk_units(0, 3, ready=25000)
    qk_units(1, 3, ready=25000)
    att_units(0, 1)
    v_units(2, ready=28000)
    att_units(1, 1)
    F("tp1", lambda: emit_tp(1), 450, after=("att01", "att11"))
    wo_units(1)
    att_units(0, 2)
    v_units(3, ready=31000)
    att_units(1, 2)
    F("tp2", lambda: emit_tp(2), 450, after=("att02", "att12"))
    wo_units(2)
    att_units(0, 3)
    att_units(1, 0)
    F("tp0", lambda: emit_tp(0), 450, after=("att00", "att10"))
    wo_units(0)

    done_rows = set()
    clock = [8500.0]  # estimated Act-stream time at the emission point

    def flush(group):
        last = max((i for i, f in enumerate(fillers) if f[0] == group), default=-1)
        for f in fillers[: last + 1]:
            if not f[5]:
                f[5] = True
                f[1]()

    def group_done(g):
        return all(f[5] for f in fillers if f[0] == g)

    def pace(budget_ns):
        for f in fillers:
            if budget_ns <= 0:
                return
            if f[5]:
                continue
            g, thunk, ns, req, ready, _, after = f
            if req is not None and req not in done_rows:
                return
            if ready > clock[0] or not all(group_done(a) for a in after):
                continue
            f[5] = True
            thunk()
            budget_ns -= ns

    rows = [(0, 0, "qk01"), (0, 1, "qk10"), (1, 1, "qk02"), (0, 2, "qk12"),
            (1, 2, "qk03"), (0, 3, "qk13"), (1, 0, None)]
    emit_q_half(0, 0)
    emit_k_half(0, 0)
    for j, qb, prefetch in rows:
        pts = []
        row_pts[(j, qb)] = pts
        nkt = 4 * (qb + 1)
        for kt in range(nkt):
            r = kt - 4 * qb
            off = 128 * r if r >= 1 else 0
            pts.append(emit_score(j, qb, kt))
            if prefetch and kt == max(0, nkt - 3):
                flush(prefetch)
            if r < 0 and kt in SCH_KTS and qb >= SCH_MIN_QB:
                act_ns = 0.0
            else:
                act_ns = (512 - off) * 2 * 0.833 + 160
            sc_pe = (512 - off) * 2 * 0.5 * PE_CYC
            clock[0] += act_ns
            pace(PACE_F * act_ns - sc_pe)
        done_rows.add((j, qb))

    # final row (1, 3): emit all scores (exp stream uninterrupted), then the
    # per-qt att -> tp -> wo chains stage-pipelined; evacs after the last exp
    # ride the Act engine
    pts = []
    row_pts[(1, 3)] = pts
    av_tiles[(1, 3)] = av_ps.tile([128, 8, 128], F32, tag="av", name="av1_3")
    for kt in range(16):
        r = kt - 12
        off = 128 * r if r >= 1 else 0
        pts.append(emit_score(1, 3, kt))
        if r < 0 and kt in SCH_KTS and 3 >= SCH_MIN_QB:
            act_ns = 0.0
        else:
            act_ns = (512 - off) * 2 * 0.833 + 160
        sc_pe = (512 - off) * 2 * 0.5 * PE_CYC
        clock[0] += act_ns
        pace(PACE_F * act_ns - sc_pe)
    for qt in range(4):
        emit_att_qt(1, 3, qt)
        if qt >= 1:
            emit_tp_qt(12 + qt - 1, on_act=True)
            emit_wo_half(12 + qt - 1, 0, on_act=False)
            emit_wo_half(12 + qt - 1, 1, on_act=True)
    emit_tp_qt(15, on_act=True)
    emit_wo_half(15, 0, on_act=False)
    emit_wo_half(15, 1, on_act=True)
    for f in fillers:
        if not f[5]:
            f[5] = True
            f[1]()

    ctx.close()


def make_in_maps(x, Wq, bq, Wk, bk, Wv, bv, Wo):
    import ml_dtypes
    i = np.arange(128)[:, None]
    jj = np.arange(128)[None, :]
    mask = (i <= jj).astype(np.float16)
    eye = np.eye(128, dtype=np.float16)
    in_maps = []
    f8 = ml_dtypes.float8_e4m3
    xTb = [np.ascontiguousarray(x[b].T.astype(np.float16)) for b in range(2)]
    x8b = [np.ascontiguousarray(x[b].T.astype(f8)) for b in range(2)]
    for c in range(8):
        b, hg = divmod(c, 4)
        sl = slice(hg * S, (hg + 1) * S)
        bqc = bq[sl].astype(np.float32).reshape(2, 128).T  # [128, fb]
        in_maps.append(
            {
                "xT": xTb[b],
                "x8": x8b[b],
                "wq8": np.ascontiguousarray(Wq[:, sl].astype(f8)),
                "wk8": np.ascontiguousarray(Wk[:, sl].astype(f8)),
                "wv": np.ascontiguousarray(Wv[:, sl].astype(np.float16)),
                "wo": np.ascontiguousarray(Wo[sl, :].astype(np.float16)),
                "bq": np.ascontiguousarray(bqc),
                "mask": mask,
                "eye": eye,
            }
        )
    return in_maps


_NC_CACHE = None


def _get_nc():
    global _NC_CACHE
    if _NC_CACHE is None:
        _NC_CACHE = build_nc()
    return _NC_CACHE


def _run(x, Wq, bq, Wk, bk, Wv, bv, Wo, bo, trace=False, **spmd_kwargs):
    nc = _get_nc()
    x, Wq, bq, Wk, bk, Wv, bv, Wo, bo = (
        np.asarray(a) for a in (x, Wq, bq, Wk, bk, Wv, bv, Wo, bo)
    )
    in_maps = make_in_maps(x, Wq, bq, Wk, bk, Wv, bv, Wo)
    res = run_bass_kernel_spmd(
        nc, in_maps, core_ids=list(range(8)), trace=trace, **spmd_kwargs
    )
    # bv and bo fold into one exact host-side bias: out += bo + bv @ Wo
    bias = (bo.astype(np.float64) + bv.astype(np.float64) @ Wo.astype(np.float64))
    out = np.zeros((2, T, E), dtype=np.float32)
    for c in range(8):
        out[c // 4] += res.results[c]["out"]
    out += bias.astype(np.float32)[None, None, :]
    return out, res


def kernel(x, Wq, bq, Wk, bk, Wv, bv, Wo, bo):
    out, _ = _run(x, Wq, bq, Wk, bk, Wv, bv, Wo, bo)
    return out
